# revision 7
# baseline (speedup 1.0000x reference)
"""Trainium2 Bass kernel for nn_E2ECompressedGEBDModel (SPoS GEBD model).

Full pipeline per window: 2-layer LSTM (seq len 17) -> per-group cosine
self-similarity (4 groups x 17x17) -> 4x (5x5 conv + BN + ReLU) -> global
mean pool.  Data-parallel over the 1024 independent windows: core c gets
SPoS offset c (128 windows).  Window extraction / scatter-back are pure
index gathers done host-side; all FLOPs run on-device.

conv1-3 run as fp8(e4m3) DoubleRow matmuls (K=256 per instruction, 2x
tensor throughput) with error-feedback (snake-order) weight rounding to
keep quantization error ~0.8%.  Boundary taps only compute their valid
output sub-rectangle (padding-skip).  conv0 uses a 100-row fp16 im2col
built by DMA.

Self-contained: hardcodes all shapes; does not read ./reference.py etc.
"""

import math
import sys
import types

import numpy as np
import ml_dtypes

K = 8
DIM = 256
GROUP = 4
B, T = 4, 256
NW = T // K              # 32 windows per (batch, offset)
L = 2 * K + 1            # 17 sequence length
NCORES = 8
NSAMP = B * NW           # 128 windows per core
H = DIM
GD = DIM // GROUP        # 64

E4 = ml_dtypes.float8_e4m3


def _install_ntff_hook():
    """The agent image's antenv lacks axon_hooks; synthesize it so
    run_bass_kernel_spmd(trace=True) can capture NTFF profiles."""
    if "antenv.axon_hooks" in sys.modules:
        return
    import antenv

    hooks_mod = types.ModuleType("antenv.axon_hooks")
    _hook = [None]
    hooks_mod.set_axon_ntff_profile_hook = lambda h: _hook.__setitem__(0, h)
    hooks_mod.get_axon_ntff_profile_hook = lambda: _hook[0]
    sys.modules["antenv.axon_hooks"] = hooks_mod
    antenv.axon_hooks = hooks_mod
    try:
        from trn_agent_boot.trn_boot import _ntff_profile_via_ctypes

        hooks_mod.set_axon_ntff_profile_hook(
            _ntff_profile_via_ctypes("/opt/axon/libaxon_pjrt.so")
        )
    except Exception:
        pass


# conv tap -> valid output sub-rectangle (r0,r1,c0,c1) and source origin
# (sr,sc) in the unpadded 17x17 input.  tap (dy,dx) contributes to output
# (r,c) from input (r+dy-2, c+dx-2).
def _tap_geom(tau):
    dy, dx = divmod(tau, 5)
    r0, r1 = max(0, 2 - dy), 17 + min(0, 2 - dy)
    c0, c1 = max(0, 2 - dx), 17 + min(0, 2 - dx)
    return r0, r1, c0, c1, max(dy - 2, 0), max(dx - 2, 0)


TAU_ORDER = [12] + [t for t in range(25) if t != 12]   # full-window tap first


def build_program(nsamp=NSAMP, stop_after=None):
    """Build + compile the per-core Bass program (SPMD, identical on all
    cores)."""
    import concourse.bass as bass
    import concourse.mybir as mybir
    import concourse.tile as tile
    from concourse import bacc
    from concourse.masks import make_identity

    dt = mybir.dt
    f32, f16, f8 = dt.float32, dt.float16, dt.float8e4
    AF = mybir.ActivationFunctionType
    PM = mybir.MatmulPerfMode
    NS = nsamp
    ts = bass.ts

    nc = bacc.Bacc("TRN2", target_bir_lowering=False, debug=False,
                   num_devices=NCORES)

    # ---- DRAM I/O --------------------------------------------------------
    d_xcatT = nc.dram_tensor("xcatT", [256, L * NS], f16, kind="ExternalInput")
    d_wc = [nc.dram_tensor(f"wc{l}", [512, 1024], f16, kind="ExternalInput")
            for l in range(2)]
    d_bias = [nc.dram_tensor(f"bias{l}", [1, 1024], f16, kind="ExternalInput")
              for l in range(2)]
    d_w0 = nc.dram_tensor("w0", [100, 2 * 128], f16, kind="ExternalInput")
    d_wconv = [nc.dram_tensor(f"w{l}", [256, 25 * 2 * 128], f8,
                              kind="ExternalInput") for l in (1, 2, 3)]
    d_bn = nc.dram_tensor("bn", [256, 8], f32, kind="ExternalInput")
    d_out = nc.dram_tensor("hout", [256, NS], f32, kind="ExternalOutput")

    with tile.TileContext(nc) as tc:
        wpool = tc.alloc_tile_pool(name="weights", bufs=1)
        work = tc.alloc_tile_pool(name="work", bufs=1)

        # ---- persistent SBUF tensors ------------------------------------
        sb_wc = wpool.tile([128, 2, 4, 1024], f16, tag="wc")
        for l in range(2):
            nc.sync.dma_start(
                out=sb_wc[:, l],
                in_=d_wc[l].ap().rearrange("(a p) n -> p a n", p=128))
        sb_bias = wpool.tile([33, 1024], f16, tag="bias")
        for l in range(2):
            nc.sync.dma_start(out=sb_bias[32 * l:32 * l + 1, :],
                              in_=d_bias[l].ap())
        sb_w0 = wpool.tile([128, 2, 128], f16, tag="w0")
        nc.sync.dma_start(
            out=sb_w0[0:100],
            in_=d_w0.ap().rearrange("p (mt m) -> p mt m", mt=2))
        sb_bn = wpool.tile([128, 2, 8], f32, tag="bn")
        nc.sync.dma_start(out=sb_bn,
                          in_=d_bn.ap().rearrange("(a p) n -> p a n", p=128))
        sb_ones = wpool.tile([33, 128], f16, tag="ones")
        nc.vector.memset(sb_ones[0:1], 1.0)
        nc.vector.memset(sb_ones[32:33], 1.0)
        sb_ident = wpool.tile([128, 128], f32, tag="ident")
        make_identity(nc, sb_ident)

        # LSTM state
        sb_h = work.tile([128, 2, 256], f32, tag="h")        # [s, layer, hid]
        sb_c = work.tile([128, 2, 256], f32, tag="c")
        nc.vector.memset(sb_h, 0.0)
        nc.vector.memset(sb_c, 0.0)
        h0T = work.tile([128, 2, NS, L], f16, tag="h0T")     # [hid_p, kt, s, t]
        hhT = work.tile([128, 2, 2, NS], f16, tag="hhT")     # [hid_p, t%2, kt, s]
        hnT = work.tile([128, 2, NS, L], f16, tag="hnT")     # normalized h1^T
        sb_sig = work.tile([128, 2, 768], f32, tag="sig")
        sb_gg = work.tile([128, 2, 256], f32, tag="gg")
        sb_ig = work.tile([128, 2, 256], f32, tag="ig")
        sb_tc = work.tile([128, 2, 256], f32, tag="tc")
        sb_hn = work.tile([128, 256], f32, tag="hn")
        sb_sq = work.tile([128, 256], f32, tag="sq")
        sb_ss = work.tile([128, 4], f32, tag="ss")
        sb_sr = work.tile([128, 4], f32, tag="sr")
        sb_rn = work.tile([128, 4], f32, tag="rn")
        sb_eps = work.tile([128, 1], f32, tag="eps")
        nc.vector.memset(sb_eps, 1e-8)

        # conv stage persistent buffers
        zp = work.tile([128, 2, 128], f16, tag="zp")
        nc.vector.memset(zp, 0.0)
        s1 = [work.tile([128, 17], f16, tag=f"s1_{i}", name=f"s1_{i}")
              for i in range(2)]
        sim_pad = work.tile([128, 2, 448], f16, tag="sim_pad")  # [g(4), pp, 21*21+7]
        nc.vector.memset(sim_pad, 0.0)
        ic16 = work.tile([128, 2, 368], f16, tag="ic16")        # [(dx,g,dy), pp, 357]
        # unpadded fp8 activations [ch_p, pp, kt, 17*17(+pad)]
        act = [work.tile([128, 2, 2, 304], f8, tag=f"act{i}", name=f"act{i}")
               for i in range(3)]
        scratch = work.tile([128, 2, 289], f32, tag="scratch")
        hout_sb = work.tile([128, 2, NS], f32, tag="hout_sb")
        nc.vector.memset(hout_sb, 0.0)

        # ================= LSTM (2 layers, interleaved) ===================
        xcpool = tc.alloc_tile_pool(name="xcpool", bufs=1)
        sb_xcatT = xcpool.tile([128, 2, L * NS], f16, tag="xcatT")
        nc.sync.dma_start(out=sb_xcatT,
                          in_=d_xcatT.ap().rearrange("(a p) n -> p a n", p=128))
        psz = tc.alloc_tile_pool(name="psz", bufs=2, space="PSUM")
        pst = tc.alloc_tile_pool(name="pst", bufs=4, space="PSUM")

        def lstm_step(layer, t):
            ps = psz.tile([128, 1024], f32, tag="z")
            # bias (k=1 matmul with ones row) — also opens the accum group
            for nh in range(2):
                nc.tensor.matmul(
                    ps[0:NS, ts(nh, 512)],
                    sb_ones[32 * layer:32 * layer + 1, 0:NS],
                    sb_bias[32 * layer:32 * layer + 1, ts(nh, 512)],
                    start=True, stop=False)
            # input contribution
            for kt in range(2):
                if layer == 0:
                    lhsT = sb_xcatT[:, kt, t * NS:(t + 1) * NS]
                else:
                    lhsT = h0T[:, kt, :, t]
                for nh in range(2):
                    nc.tensor.matmul(
                        ps[0:NS, ts(nh, 512)], lhsT,
                        sb_wc[:, layer, kt, ts(nh, 512)],
                        start=False, stop=(t == 0 and kt == 1))
            # recurrent contribution (h_{-1} = 0 -> skip at t=0)
            if t > 0:
                for kt in range(2):
                    if layer == 0:
                        lhsT = h0T[:, kt, :, t - 1]
                    else:
                        lhsT = hhT[:, (t - 1) % 2, kt, :]
                    for nh in range(2):
                        nc.tensor.matmul(
                            ps[0:NS, ts(nh, 512)], lhsT,
                            sb_wc[:, layer, 2 + kt, ts(nh, 512)],
                            start=False, stop=(kt == 1))
            # gates: layout [i(0:256) f(256:512) o(512:768) | g(768:1024)]
            sig = sb_sig[0:NS, layer]
            nc.scalar.activation(sig, ps[0:NS, 0:768], AF.Sigmoid)
            nc.scalar.activation(sb_gg[0:NS, layer], ps[0:NS, 768:1024], AF.Tanh)
            c_ = sb_c[0:NS, layer]
            h_ = sb_h[0:NS, layer]
            nc.vector.tensor_mul(sb_ig[0:NS, layer], sig[:, 0:256],
                                 sb_gg[0:NS, layer])
            if t > 0:
                nc.vector.tensor_mul(c_, sig[:, 256:512], c_)
                nc.vector.tensor_add(c_, c_, sb_ig[0:NS, layer])
            else:
                nc.vector.tensor_copy(c_, sb_ig[0:NS, layer])
            nc.scalar.activation(sb_tc[0:NS, layer], c_, AF.Tanh)
            nc.vector.tensor_mul(h_, sig[:, 512:768], sb_tc[0:NS, layer])
            # transpose h_t (raw) for recurrence / next-layer input
            for kt in range(2):
                pt = pst.tile([128, NS], f32, tag="tr")
                nc.tensor.transpose(pt, h_[:, ts(kt, 128)],
                                    sb_ident[0:NS, 0:NS])
                dest = h0T[:, kt, :, t] if layer == 0 else hhT[:, t % 2, kt, :]
                nc.vector.tensor_copy(dest, pt)
            if layer == 1:
                # normalize per similarity group, transpose into hnT
                nc.vector.tensor_mul(sb_sq[0:NS], h_, h_)
                nc.vector.reduce_sum(
                    sb_ss[0:NS],
                    sb_sq[0:NS].rearrange("p (g d) -> p g d", g=4),
                    axis=mybir.AxisListType.X)
                nc.scalar.activation(sb_sr[0:NS], sb_ss[0:NS], AF.Sqrt,
                                     bias=sb_eps[0:NS])
                nc.vector.reciprocal(sb_rn[0:NS], sb_sr[0:NS])
                for g in range(4):
                    nc.vector.tensor_scalar_mul(
                        sb_hn[0:NS, ts(g, GD)], h_[:, ts(g, GD)],
                        sb_rn[0:NS, g:g + 1])
                for kt in range(2):
                    pt = pst.tile([128, NS], f32, tag="tr")
                    nc.tensor.transpose(pt, sb_hn[0:NS, ts(kt, 128)],
                                        sb_ident[0:NS, 0:NS])
                    nc.vector.tensor_copy(hnT[:, kt, :, t], pt)

        # software-pipeline the two layers: L1 runs one step behind L0
        lstm_step(0, 0)
        lstm_step(0, 1)
        for t in range(2, L + 2):
            if t < L:
                lstm_step(0, t)
            lstm_step(1, t - 2)
        pst.release()
        psz.release()
        xcpool.release()

        # ================= similarity + convs, per sample =================
        cwpool = tc.alloc_tile_pool(name="cwpool", bufs=1)
        sb_wconv = cwpool.tile([128, 3, 2, 25, 2, 128], f8, tag="wconv")
        for i in range(3):
            nc.sync.dma_start(
                out=sb_wconv[:, i],
                in_=d_wconv[i].ap().rearrange("(a p) n -> p a n", p=128))
        psim = tc.alloc_tile_pool(name="psim", bufs=2, space="PSUM")
        pcv = tc.alloc_tile_pool(name="pcv", bufs=1, space="PSUM")
        pc_t = [pcv.tile([128, 2, 512], f32, tag=f"pc{i}", name=f"pc{i}")
                for i in range(2)]

        def sim_stage(s):
            pp = s % 2
            # stationary zp: column block [32g : 32g+17] holds group g's
            # normalized vectors (rows = hidden slice of that group, zeros
            # elsewhere) -> one matmul accumulation group computes all 4
            # group-dot blocks into psum partitions [32g:32g+17].
            for g in range(4):
                kt, ko = g // 2, (g % 2) * GD
                nc.vector.tensor_copy(
                    zp[ko:ko + GD, kt, 32 * g:32 * g + 17],
                    hnT[ko:ko + GD, kt, s, :])
            ps = psim.tile([128, 17], f32, tag="psim")
            for kt in range(2):
                nc.tensor.matmul(ps, zp[:, kt, :], hnT[:, kt, s, :],
                                 start=(kt == 0), stop=(kt == 1))
            nc.vector.tensor_copy(s1[pp], ps)
            # regroup [32g+i, j] -> padded image [g, i(row), j(col)]
            for g in range(4):
                dst = sim_pad[g:g + 1, pp, 0:441].rearrange(
                    "p (r c) -> p r c", c=21)[:, 2:19, 2:19]
                nc.sync.dma_start(out=dst, in_=s1[pp][32 * g:32 * g + 17, :])
            # im2col gather: row (dx,g,dy) = 357-run sim_pad[g, dy*21+dx :][:357]
            # (contiguous run keeps each DMA within the 3-dim balancer limit;
            # the matmul below views it as 17 rows of pitch 21).
            sp = sim_pad[0:4, pp, 0:441]
            for dx in range(5):
                src = bass.AP(tensor=sp.tensor, offset=sp.offset + dx,
                              ap=[sp.ap[0], [21, 5], [1, 357]])
                nc.sync.dma_start(out=ic16[dx * 20:(dx + 1) * 20, pp, 0:357],
                                  in_=src)

        def conv0_stage(s):
            pp = s % 2
            icv = ic16[0:100, pp, 0:357].rearrange(
                "p (r c) -> p r c", c=21)[:, :, 0:17]
            for mt in range(2):
                nc.tensor.matmul(pc_t[pp][:, mt, 0:289],
                                 sb_w0[0:100, mt, :],
                                 icv,
                                 start=True, stop=True)
                nc.scalar.activation(
                    act[0][:, pp, mt, 0:289], pc_t[pp][:, mt, 0:289],
                    AF.Relu, scale=sb_bn[:, mt, 0:1], bias=sb_bn[:, mt, 1:2])

        def conv_stage(s, lyr):
            """lyr in 1..3: act[lyr-1] -> act[lyr] (or pooled out).
            fp8 DoubleRow matmuls, padding-skip sub-rectangles."""
            pp = s % 2
            pc = pc_t[pp]
            src = act[lyr - 1][:, pp, :, 0:289].rearrange(
                "p a (r c) -> p a r c", c=17)
            for mt in range(2):
                out289 = pc[:, mt, 0:289].rearrange("p (r c) -> p r c", c=17)
                for tau in TAU_ORDER:
                    r0, r1, c0, c1, sr, sc = _tap_geom(tau)
                    nc.tensor.matmul(
                        out289[:, r0:r1, c0:c1],
                        sb_wconv[:, lyr - 1, :, tau, mt, :],
                        src[:, :, sr:sr + (r1 - r0), sc:sc + (c1 - c0)],
                        start=(tau == 12), stop=(tau == TAU_ORDER[-1]),
                        perf_mode=PM.DoubleRow)
                if lyr < 3:
                    nc.scalar.activation(
                        act[lyr][:, pp, mt, 0:289], pc[:, mt, 0:289],
                        AF.Relu, scale=sb_bn[:, mt, 2 * lyr:2 * lyr + 1],
                        bias=sb_bn[:, mt, 2 * lyr + 1:2 * lyr + 2])
                else:
                    nc.scalar.activation(
                        scratch[:, mt], pc[:, mt, 0:289], AF.Relu,
                        scale=sb_bn[:, mt, 6:7], bias=sb_bn[:, mt, 7:8],
                        accum_out=hout_sb[:, mt, s:s + 1])

        # pair-pipelined emission
        if stop_after != "lstm":
            last = {"sim": 0, "conv0": 0, "conv1": 1, "conv2": 2}.get(
                stop_after, 3)
            for p in range(0, nsamp, 2):
                pair = [p] if p + 1 >= nsamp else [p, p + 1]
                for s in pair:
                    sim_stage(s)
                if stop_after != "sim":
                    for s in pair:
                        conv0_stage(s)
                for lyr in (1, 2, 3):
                    if lyr > last:
                        break
                    for s in pair:
                        conv_stage(s, lyr)

        nc.sync.dma_start(
            out=d_out.ap().rearrange("(a p) n -> p a n", p=128),
            in_=hout_sb[:, :, 0:NS])

        pcv.release()
        psim.release()
        cwpool.release()
        work.release()
        wpool.release()

    nc.compile()
    return nc


# ======================= host-side preparation ===========================

def make_xcat(x):
    """Window extraction, identical to the reference (pL == T case)."""
    x = np.asarray(x, np.float32)
    lefts, rights, mids = [], [], []
    for offset in range(K):
        s = K - offset
        left = np.concatenate(
            [np.repeat(x[:, :, :1], s, axis=2), x[:, :, :-s]], axis=2)
        r = offset + 1
        right = np.concatenate(
            [x[:, :, r:], np.repeat(x[:, :, -1:], r, axis=2)], axis=2)
        lefts.append(left.reshape(B, DIM, NW, K).transpose(0, 2, 3, 1)
                     .reshape(B * NW, K, DIM))
        rights.append(right.reshape(B, DIM, NW, K).transpose(0, 2, 3, 1)
                      .reshape(B * NW, K, DIM))
        mids.append(x[:, :, offset::K].transpose(0, 2, 1)
                    .reshape(B * NW, 1, DIM))
    left_seq = np.concatenate(lefts, axis=0)
    right_seq = np.concatenate(rights, axis=0)
    mid_seq = np.concatenate(mids, axis=0)
    return np.concatenate([left_seq, mid_seq, right_seq], axis=1)  # (1024,17,256)


def _quant_ef_snake(w, target=224.0):
    """e4m3 quantization with snake-order (dy,dx) error feedback per (o,i).
    Returns (fp8 array same shape, scale s applied: wq ~ w*s)."""
    w = np.asarray(w, np.float32)
    mx = max(np.abs(w).max(), 1e-20)
    s = float(2.0 ** np.floor(np.log2(target / mx)))
    ws = w * s
    out = np.zeros(w.shape, E4)
    carry = np.zeros(w.shape[:2], np.float32)
    idx = []
    for dy in range(5):
        rng = range(5) if dy % 2 == 0 else range(4, -1, -1)
        idx += [(dy, dx) for dx in rng]
    for dy, dx in idx:
        v = ws[:, :, dy, dx] + carry
        qv = v.astype(E4)
        carry = v - qv.astype(np.float32)
        out[:, :, dy, dx] = qv
    return out, s


def prep_weights(inp):
    """Host-side reorder of parameters into the device layouts."""
    g = {}
    perm = np.concatenate([np.arange(0, 256), np.arange(256, 512),
                           np.arange(768, 1024), np.arange(512, 768)])
    for l in range(2):
        wih = np.asarray(inp[f"w_ih{l}"], np.float32)[perm]
        whh = np.asarray(inp[f"w_hh{l}"], np.float32)[perm]
        g[f"wc{l}"] = np.ascontiguousarray(
            np.vstack([wih.T, whh.T]), dtype=np.float16)        # (512,1024)
        g[f"bias{l}"] = np.ascontiguousarray(
            (np.asarray(inp[f"b_ih{l}"], np.float32)
             + np.asarray(inp[f"b_hh{l}"], np.float32))[perm][None, :],
            dtype=np.float16)
    # conv0 layout: rows (dx,g,dy)=100, cols (mt, m)
    w0 = np.asarray(inp["conv0_w"], np.float32)                 # (256,4,5,5)
    t = w0.transpose(3, 1, 2, 0).reshape(100, 256)              # (dx,g,dy),(cout)
    g["w0"] = np.ascontiguousarray(t, dtype=np.float16)
    wscale = np.zeros(4, np.float32)
    wscale[0] = 1.0
    for i, name in enumerate(("conv1_w", "conv2_w", "conv3_w")):
        w = np.asarray(inp[name], np.float32)                   # (256,256,5,5)
        wq, s = _quant_ef_snake(w)
        wscale[i + 1] = s
        t = wq.transpose(1, 2, 3, 0)        # (cin, dy, dx, cout) fp8
        t = t.reshape(2, 128, 25, 2, 128)   # (kt, p, tau, mt, m)
        g[f"w{i + 1}"] = np.ascontiguousarray(t.reshape(256, 25 * 2 * 128))
    bn = np.zeros((256, 8), np.float32)
    for i in range(4):
        s = np.asarray(inp[f"bn{i}_s"], np.float32) / wscale[i]
        b = np.asarray(inp[f"bn{i}_b"], np.float32)
        if i == 3:
            s = s / 289.0
            b = b / 289.0
        bn[:, 2 * i] = s
        bn[:, 2 * i + 1] = b
    g["bn"] = bn
    return g


_CACHE = {}


def kernel(**inputs):
    _install_ntff_hook()
    from concourse.bass_utils import run_bass_kernel_spmd

    if "nc" not in _CACHE:
        _CACHE["nc"] = build_program(NSAMP)
    nc = _CACHE["nc"]

    shared = prep_weights(inputs)
    xcat = make_xcat(inputs["x"])           # (1024, 17, 256)
    in_maps = []
    for c in range(NCORES):
        xc = xcat[c * NSAMP:(c + 1) * NSAMP]            # (128, 17, 256)
        xcT = np.ascontiguousarray(
            xc.transpose(2, 1, 0).reshape(256, L * NSAMP), dtype=np.float16)
        m = dict(shared)
        m["xcatT"] = xcT
        in_maps.append(m)

    res = run_bass_kernel_spmd(nc, in_maps, core_ids=list(range(NCORES)))
    out = np.zeros((B, DIM, T), np.float32)
    for c in range(NCORES):
        hc = res.results[c]["hout"].T                   # (128, 256)
        out[:, :, c::K] = hc.reshape(B, NW, DIM).transpose(0, 2, 1)
    return out


# revision 11
# speedup vs baseline: 1.0521x; 1.0521x over previous
"""Trainium2 Bass kernel for nn_E2ECompressedGEBDModel (SPoS GEBD model).

Full pipeline per window: 2-layer LSTM (seq len 17) -> per-group cosine
self-similarity (4 groups x 17x17) -> 4x (5x5 conv + BN + ReLU) -> global
mean pool.  Data-parallel over the 1024 independent windows: core c gets
SPoS offset c (128 windows).  Window extraction / scatter-back are pure
index gathers done host-side; all FLOPs run on-device.

conv1-3 run as fp8(e4m3) DoubleRow matmuls (K=256 per instruction, 2x
tensor throughput) with error-feedback (snake-order) weight rounding to
keep quantization error ~0.8%.  Boundary taps only compute their valid
output sub-rectangle (padding-skip).  conv0 uses a 100-row fp16 im2col
built by DMA.

Self-contained: hardcodes all shapes; does not read ./reference.py etc.
"""

import math
import sys
import types

import numpy as np
import ml_dtypes

K = 8
DIM = 256
GROUP = 4
B, T = 4, 256
NW = T // K              # 32 windows per (batch, offset)
L = 2 * K + 1            # 17 sequence length
NCORES = 8
NSAMP = B * NW           # 128 windows per core
H = DIM
GD = DIM // GROUP        # 64

E4 = ml_dtypes.float8_e4m3


def _install_ntff_hook():
    """The agent image's antenv lacks axon_hooks; synthesize it so
    run_bass_kernel_spmd(trace=True) can capture NTFF profiles."""
    if "antenv.axon_hooks" in sys.modules:
        return
    import antenv

    hooks_mod = types.ModuleType("antenv.axon_hooks")
    _hook = [None]
    hooks_mod.set_axon_ntff_profile_hook = lambda h: _hook.__setitem__(0, h)
    hooks_mod.get_axon_ntff_profile_hook = lambda: _hook[0]
    sys.modules["antenv.axon_hooks"] = hooks_mod
    antenv.axon_hooks = hooks_mod
    try:
        from trn_agent_boot.trn_boot import _ntff_profile_via_ctypes

        hooks_mod.set_axon_ntff_profile_hook(
            _ntff_profile_via_ctypes("/opt/axon/libaxon_pjrt.so")
        )
    except Exception:
        pass


# conv tap -> valid output sub-rectangle (r0,r1,c0,c1) and source origin
# (sr,sc) in the unpadded 17x17 input.  tap (dy,dx) contributes to output
# (r,c) from input (r+dy-2, c+dx-2).
def _tap_geom(tau):
    dy, dx = divmod(tau, 5)
    r0, r1 = max(0, 2 - dy), 17 + min(0, 2 - dy)
    c0, c1 = max(0, 2 - dx), 17 + min(0, 2 - dx)
    return r0, r1, c0, c1, max(dy - 2, 0), max(dx - 2, 0)


TAU_ORDER = [12] + [t for t in range(25) if t != 12]   # full-window tap first


def build_program(nsamp=NSAMP, stop_after=None):
    """Build + compile the per-core Bass program (SPMD, identical on all
    cores)."""
    import concourse.bass as bass
    import concourse.mybir as mybir
    import concourse.tile as tile
    from concourse import bacc
    from concourse.masks import make_identity

    dt = mybir.dt
    f32, f16, f8 = dt.float32, dt.float16, dt.float8e4
    AF = mybir.ActivationFunctionType
    PM = mybir.MatmulPerfMode
    NS = nsamp
    ts = bass.ts

    nc = bacc.Bacc("TRN2", target_bir_lowering=False, debug=False,
                   num_devices=NCORES)

    # ---- DRAM I/O --------------------------------------------------------
    d_xcatT = nc.dram_tensor("xcatT", [256, L * NS], f16, kind="ExternalInput")
    d_wc = [nc.dram_tensor(f"wc{l}", [512, 1024], f16, kind="ExternalInput")
            for l in range(2)]
    d_bias = [nc.dram_tensor(f"bias{l}", [1, 1024], f16, kind="ExternalInput")
              for l in range(2)]
    d_w0 = nc.dram_tensor("w0", [100, 2 * 128], f16, kind="ExternalInput")
    d_wconv = [nc.dram_tensor(f"w{l}", [256, 25 * 2 * 128], f8,
                              kind="ExternalInput") for l in (1, 2, 3)]
    d_bn = nc.dram_tensor("bn", [256, 8], f32, kind="ExternalInput")
    d_out = nc.dram_tensor("hout", [256, NS], f32, kind="ExternalOutput")

    with tile.TileContext(nc) as tc:
        wpool = tc.alloc_tile_pool(name="weights", bufs=1)
        work = tc.alloc_tile_pool(name="work", bufs=1)

        # ---- persistent SBUF tensors ------------------------------------
        sb_wc = wpool.tile([128, 2, 4, 1024], f16, tag="wc")
        for l in range(2):
            nc.sync.dma_start(
                out=sb_wc[:, l],
                in_=d_wc[l].ap().rearrange("(a p) n -> p a n", p=128))
        sb_bias = wpool.tile([33, 1024], f16, tag="bias")
        for l in range(2):
            nc.sync.dma_start(out=sb_bias[32 * l:32 * l + 1, :],
                              in_=d_bias[l].ap())
        sb_w0 = wpool.tile([128, 2, 128], f16, tag="w0")
        nc.sync.dma_start(
            out=sb_w0[0:100],
            in_=d_w0.ap().rearrange("p (mt m) -> p mt m", mt=2))
        sb_bn = wpool.tile([128, 2, 8], f32, tag="bn")
        nc.sync.dma_start(out=sb_bn,
                          in_=d_bn.ap().rearrange("(a p) n -> p a n", p=128))
        sb_ones = wpool.tile([33, 128], f16, tag="ones")
        nc.vector.memset(sb_ones[0:1], 1.0)
        nc.vector.memset(sb_ones[32:33], 1.0)
        sb_ident = wpool.tile([128, 128], f32, tag="ident")
        make_identity(nc, sb_ident)

        # LSTM state
        sb_h = work.tile([128, 2, 256], f32, tag="h")        # [s, layer, hid]
        sb_c = work.tile([128, 2, 256], f32, tag="c")
        nc.vector.memset(sb_h, 0.0)
        nc.vector.memset(sb_c, 0.0)
        h0T = work.tile([128, 2, NS, L], f16, tag="h0T")     # [hid_p, kt, s, t]
        hhT = work.tile([128, 2, 2, NS], f16, tag="hhT")     # [hid_p, t%2, kt, s]
        hnT = work.tile([128, 2, NS, L], f16, tag="hnT")     # normalized h1^T
        sb_sig = work.tile([128, 2, 768], f32, tag="sig")
        sb_gg = work.tile([128, 2, 256], f32, tag="gg")
        sb_ig = work.tile([128, 2, 256], f32, tag="ig")
        sb_tc = work.tile([128, 2, 256], f32, tag="tc")
        sb_hn = work.tile([128, 256], f32, tag="hn")
        sb_sq = work.tile([128, 256], f32, tag="sq")
        sb_ss = work.tile([128, 4], f32, tag="ss")
        sb_sr = work.tile([128, 4], f32, tag="sr")
        sb_rn = work.tile([128, 4], f32, tag="rn")
        sb_eps = work.tile([128, 1], f32, tag="eps")
        nc.vector.memset(sb_eps, 1e-8)

        # conv stage persistent buffers (sim front-end runs 4 samples ahead,
        # conv0 2 samples ahead of conv1-3 -> 4-slot buffers)
        zp = work.tile([128, 2, 2, 128], f16, tag="zp")         # [hid, slot, kt, col]
        nc.vector.memset(zp, 0.0)
        s1 = [work.tile([128, 17], f16, tag=f"s1_{i}", name=f"s1_{i}")
              for i in range(4)]
        sim_pad = work.tile([128, 4, 448], f16, tag="sim_pad")  # [g(4), slot, 21*21+7]
        nc.vector.memset(sim_pad, 0.0)
        ic16 = work.tile([128, 4, 368], f16, tag="ic16")        # [(dx,g,dy), slot, 357]
        # unpadded fp8 activations [ch_p, slot, kt, 17*17(+pad)]
        act0 = work.tile([128, 4, 2, 304], f8, tag="act0")
        act = [None,
               work.tile([128, 2, 2, 304], f8, tag="act1", name="act1"),
               work.tile([128, 2, 2, 304], f8, tag="act2", name="act2")]
        scratch = work.tile([128, 2, 289], f32, tag="scratch")
        hout_sb = work.tile([128, 2, NS], f32, tag="hout_sb")
        nc.vector.memset(hout_sb, 0.0)

        # ================= LSTM (2 layers, interleaved) ===================
        xcpool = tc.alloc_tile_pool(name="xcpool", bufs=1)
        sb_xcatT = xcpool.tile([128, 2, L * NS], f16, tag="xcatT")
        nc.sync.dma_start(out=sb_xcatT,
                          in_=d_xcatT.ap().rearrange("(a p) n -> p a n", p=128))
        psz = tc.alloc_tile_pool(name="psz", bufs=2, space="PSUM")
        pst = tc.alloc_tile_pool(name="pst", bufs=4, space="PSUM")

        def lstm_step(layer, t):
            ps = psz.tile([128, 1024], f32, tag="z")
            # bias (k=1 matmul with ones row) — also opens the accum group
            for nh in range(2):
                nc.tensor.matmul(
                    ps[0:NS, ts(nh, 512)],
                    sb_ones[32 * layer:32 * layer + 1, 0:NS],
                    sb_bias[32 * layer:32 * layer + 1, ts(nh, 512)],
                    start=True, stop=False)
            # input contribution
            for kt in range(2):
                if layer == 0:
                    lhsT = sb_xcatT[:, kt, t * NS:(t + 1) * NS]
                else:
                    lhsT = h0T[:, kt, :, t]
                for nh in range(2):
                    nc.tensor.matmul(
                        ps[0:NS, ts(nh, 512)], lhsT,
                        sb_wc[:, layer, kt, ts(nh, 512)],
                        start=False, stop=(t == 0 and kt == 1))
            # recurrent contribution (h_{-1} = 0 -> skip at t=0)
            if t > 0:
                for kt in range(2):
                    if layer == 0:
                        lhsT = h0T[:, kt, :, t - 1]
                    else:
                        lhsT = hhT[:, (t - 1) % 2, kt, :]
                    for nh in range(2):
                        nc.tensor.matmul(
                            ps[0:NS, ts(nh, 512)], lhsT,
                            sb_wc[:, layer, 2 + kt, ts(nh, 512)],
                            start=False, stop=(kt == 1))
            # gates: layout [i(0:256) f(256:512) o(512:768) | g(768:1024)]
            sig = sb_sig[0:NS, layer]
            nc.scalar.activation(sig, ps[0:NS, 0:768], AF.Sigmoid)
            nc.scalar.activation(sb_gg[0:NS, layer], ps[0:NS, 768:1024], AF.Tanh)
            c_ = sb_c[0:NS, layer]
            h_ = sb_h[0:NS, layer]
            nc.vector.tensor_mul(sb_ig[0:NS, layer], sig[:, 0:256],
                                 sb_gg[0:NS, layer])
            if t > 0:
                nc.vector.tensor_mul(c_, sig[:, 256:512], c_)
                nc.vector.tensor_add(c_, c_, sb_ig[0:NS, layer])
            else:
                nc.vector.tensor_copy(c_, sb_ig[0:NS, layer])
            nc.scalar.activation(sb_tc[0:NS, layer], c_, AF.Tanh)
            nc.vector.tensor_mul(h_, sig[:, 512:768], sb_tc[0:NS, layer])
            # transpose h_t (raw) for recurrence / next-layer input
            for kt in range(2):
                pt = pst.tile([128, NS], f32, tag="tr")
                nc.tensor.transpose(pt, h_[:, ts(kt, 128)],
                                    sb_ident[0:NS, 0:NS])
                dest = h0T[:, kt, :, t] if layer == 0 else hhT[:, t % 2, kt, :]
                nc.vector.tensor_copy(dest, pt)
            if layer == 1:
                # normalize per similarity group, transpose into hnT
                nc.vector.tensor_mul(sb_sq[0:NS], h_, h_)
                nc.vector.reduce_sum(
                    sb_ss[0:NS],
                    sb_sq[0:NS].rearrange("p (g d) -> p g d", g=4),
                    axis=mybir.AxisListType.X)
                nc.scalar.activation(sb_sr[0:NS], sb_ss[0:NS], AF.Sqrt,
                                     bias=sb_eps[0:NS])
                nc.vector.reciprocal(sb_rn[0:NS], sb_sr[0:NS])
                for g in range(4):
                    nc.vector.tensor_scalar_mul(
                        sb_hn[0:NS, ts(g, GD)], h_[:, ts(g, GD)],
                        sb_rn[0:NS, g:g + 1])
                for kt in range(2):
                    pt = pst.tile([128, NS], f32, tag="tr")
                    nc.tensor.transpose(pt, sb_hn[0:NS, ts(kt, 128)],
                                        sb_ident[0:NS, 0:NS])
                    nc.vector.tensor_copy(hnT[:, kt, :, t], pt)

        # software-pipeline the two layers: L1 runs one step behind L0
        lstm_step(0, 0)
        lstm_step(0, 1)
        for t in range(2, L + 2):
            if t < L:
                lstm_step(0, t)
            lstm_step(1, t - 2)
        pst.release()
        psz.release()
        xcpool.release()

        # ================= similarity + convs, per sample =================
        cwpool = tc.alloc_tile_pool(name="cwpool", bufs=1)
        sb_wconv = cwpool.tile([128, 3, 2, 25, 2, 128], f8, tag="wconv")
        for i in range(3):
            nc.sync.dma_start(
                out=sb_wconv[:, i],
                in_=d_wconv[i].ap().rearrange("(a p) n -> p a n", p=128))
        psim = tc.alloc_tile_pool(name="psim", bufs=1, space="PSUM")
        pcv0 = tc.alloc_tile_pool(name="pcv0", bufs=1, space="PSUM")
        pc_0 = pcv0.tile([128, 2, 512], f32, tag="pc0t")
        pcv = tc.alloc_tile_pool(name="pcv", bufs=1, space="PSUM")
        pc_t = [pcv.tile([128, 2, 512], f32, tag=f"pc{i}", name=f"pc{i}")
                for i in range(2)]

        def sim_stage(s):
            sl = s % 4
            # stationary zp: column block [32g : 32g+17] holds group g's
            # normalized vectors (rows = hidden slice of that group, zeros
            # elsewhere) -> one matmul accumulation group computes all 4
            # group-dot blocks into psum partitions [32g:32g+17].
            for g in range(4):
                kt, ko = g // 2, (g % 2) * GD
                nc.vector.tensor_copy(
                    zp[ko:ko + GD, s % 2, kt, 32 * g:32 * g + 17],
                    hnT[ko:ko + GD, kt, s, :])
            ps = psim.tile([128, 17], f32, tag="psim")
            for kt in range(2):
                nc.tensor.matmul(ps, zp[:, s % 2, kt, :], hnT[:, kt, s, :],
                                 start=(kt == 0), stop=(kt == 1))
            nc.vector.tensor_copy(s1[sl], ps)
            # regroup [32g+i, j] -> padded image [g, i(row), j(col)]
            for g in range(4):
                dst = sim_pad[g:g + 1, sl, 0:441].rearrange(
                    "p (r c) -> p r c", c=21)[:, 2:19, 2:19]
                nc.sync.dma_start(out=dst, in_=s1[sl][32 * g:32 * g + 17, :])
            # im2col gather: row (dx,g,dy) = 357-run sim_pad[g, dy*21+dx :][:357]
            # (contiguous run keeps each DMA within the 3-dim balancer limit;
            # the matmul below views it as 17 rows of pitch 21).
            sp = sim_pad[0:4, sl, 0:441]
            for dx in range(5):
                src = bass.AP(tensor=sp.tensor, offset=sp.offset + dx,
                              ap=[sp.ap[0], [21, 5], [1, 357]])
                nc.sync.dma_start(out=ic16[dx * 20:(dx + 1) * 20, sl, 0:357],
                                  in_=src)

        def conv0_stage(s):
            sl = s % 4
            icv = ic16[0:100, sl, 0:357].rearrange(
                "p (r c) -> p r c", c=21)[:, :, 0:17]
            for mt in range(2):
                nc.tensor.matmul(pc_0[:, mt, 0:289],
                                 sb_w0[0:100, mt, :],
                                 icv,
                                 start=True, stop=True)
                nc.scalar.activation(
                    act0[:, sl, mt, 0:289], pc_0[:, mt, 0:289],
                    AF.Relu, scale=sb_bn[:, mt, 0:1], bias=sb_bn[:, mt, 1:2])

        def conv_stage(s, lyr):
            """lyr in 1..3: act[lyr-1] -> act[lyr] (or pooled out).
            fp8 DoubleRow matmuls, padding-skip sub-rectangles."""
            pp = s % 2
            pc = pc_t[pp]
            if lyr == 1:
                src = act0[:, s % 4, :, 0:289].rearrange(
                    "p a (r c) -> p a r c", c=17)
            else:
                src = act[lyr - 1][:, pp, :, 0:289].rearrange(
                    "p a (r c) -> p a r c", c=17)
            for mt in range(2):
                out289 = pc[:, mt, 0:289].rearrange("p (r c) -> p r c", c=17)
                for tau in TAU_ORDER:
                    r0, r1, c0, c1, sr, sc = _tap_geom(tau)
                    nc.tensor.matmul(
                        out289[:, r0:r1, c0:c1],
                        sb_wconv[:, lyr - 1, :, tau, mt, :],
                        src[:, :, sr:sr + (r1 - r0), sc:sc + (c1 - c0)],
                        start=(tau == 12), stop=(tau == TAU_ORDER[-1]),
                        perf_mode=PM.DoubleRow)
                if lyr < 3:
                    nc.scalar.activation(
                        act[lyr][:, pp, mt, 0:289], pc[:, mt, 0:289],
                        AF.Relu, scale=sb_bn[:, mt, 2 * lyr:2 * lyr + 1],
                        bias=sb_bn[:, mt, 2 * lyr + 1:2 * lyr + 2])
                else:
                    nc.scalar.activation(
                        scratch[:, mt], pc[:, mt, 0:289], AF.Relu,
                        scale=sb_bn[:, mt, 6:7], bias=sb_bn[:, mt, 7:8],
                        accum_out=hout_sb[:, mt, s:s + 1])

        # pipelined emission: sims run 4 samples ahead, conv0 2 ahead of the
        # conv1-3 chain; the second conv0 of each window is emitted mid-
        # iteration so the shared pc_0 tile's WAR distance stays long.
        if stop_after != "lstm":
            last = {"sim": 0, "conv0": 0, "conv1": 1, "conv2": 2}.get(
                stop_after, 3)

            def do(stage, s):
                if s < nsamp:
                    stage(s)

            for s in range(4):
                do(sim_stage, s)
            if stop_after != "sim":
                for s in range(2):
                    do(conv0_stage, s)
                for p in range(0, nsamp, 2):
                    pair = [s for s in (p, p + 1) if s < nsamp]
                    do(sim_stage, p + 4)
                    do(sim_stage, p + 5)
                    do(conv0_stage, p + 2)
                    if last >= 1:
                        for s in pair:
                            conv_stage(s, 1)
                    do(conv0_stage, p + 3)
                    for lyr in (2, 3):
                        if lyr > last:
                            break
                        for s in pair:
                            conv_stage(s, lyr)

        nc.sync.dma_start(
            out=d_out.ap().rearrange("(a p) n -> p a n", p=128),
            in_=hout_sb[:, :, 0:NS])

        pcv.release()
        pcv0.release()
        psim.release()
        cwpool.release()
        work.release()
        wpool.release()

    nc.compile()
    return nc


# ======================= host-side preparation ===========================

def make_xcat(x):
    """Window extraction, identical to the reference (pL == T case)."""
    x = np.asarray(x, np.float32)
    lefts, rights, mids = [], [], []
    for offset in range(K):
        s = K - offset
        left = np.concatenate(
            [np.repeat(x[:, :, :1], s, axis=2), x[:, :, :-s]], axis=2)
        r = offset + 1
        right = np.concatenate(
            [x[:, :, r:], np.repeat(x[:, :, -1:], r, axis=2)], axis=2)
        lefts.append(left.reshape(B, DIM, NW, K).transpose(0, 2, 3, 1)
                     .reshape(B * NW, K, DIM))
        rights.append(right.reshape(B, DIM, NW, K).transpose(0, 2, 3, 1)
                      .reshape(B * NW, K, DIM))
        mids.append(x[:, :, offset::K].transpose(0, 2, 1)
                    .reshape(B * NW, 1, DIM))
    left_seq = np.concatenate(lefts, axis=0)
    right_seq = np.concatenate(rights, axis=0)
    mid_seq = np.concatenate(mids, axis=0)
    return np.concatenate([left_seq, mid_seq, right_seq], axis=1)  # (1024,17,256)


def _quant_ef_snake(w, target=224.0):
    """e4m3 quantization with snake-order (dy,dx) error feedback per (o,i).
    Returns (fp8 array same shape, scale s applied: wq ~ w*s)."""
    w = np.asarray(w, np.float32)
    mx = max(np.abs(w).max(), 1e-20)
    s = float(2.0 ** np.floor(np.log2(target / mx)))
    ws = w * s
    out = np.zeros(w.shape, E4)
    carry = np.zeros(w.shape[:2], np.float32)
    idx = []
    for dy in range(5):
        rng = range(5) if dy % 2 == 0 else range(4, -1, -1)
        idx += [(dy, dx) for dx in rng]
    for dy, dx in idx:
        v = ws[:, :, dy, dx] + carry
        qv = v.astype(E4)
        carry = v - qv.astype(np.float32)
        out[:, :, dy, dx] = qv
    return out, s


def prep_weights(inp):
    """Host-side reorder of parameters into the device layouts."""
    g = {}
    perm = np.concatenate([np.arange(0, 256), np.arange(256, 512),
                           np.arange(768, 1024), np.arange(512, 768)])
    for l in range(2):
        wih = np.asarray(inp[f"w_ih{l}"], np.float32)[perm]
        whh = np.asarray(inp[f"w_hh{l}"], np.float32)[perm]
        g[f"wc{l}"] = np.ascontiguousarray(
            np.vstack([wih.T, whh.T]), dtype=np.float16)        # (512,1024)
        g[f"bias{l}"] = np.ascontiguousarray(
            (np.asarray(inp[f"b_ih{l}"], np.float32)
             + np.asarray(inp[f"b_hh{l}"], np.float32))[perm][None, :],
            dtype=np.float16)
    # conv0 layout: rows (dx,g,dy)=100, cols (mt, m)
    w0 = np.asarray(inp["conv0_w"], np.float32)                 # (256,4,5,5)
    t = w0.transpose(3, 1, 2, 0).reshape(100, 256)              # (dx,g,dy),(cout)
    g["w0"] = np.ascontiguousarray(t, dtype=np.float16)
    wscale = np.zeros(4, np.float32)
    wscale[0] = 1.0
    for i, name in enumerate(("conv1_w", "conv2_w", "conv3_w")):
        w = np.asarray(inp[name], np.float32)                   # (256,256,5,5)
        wq, s = _quant_ef_snake(w)
        wscale[i + 1] = s
        t = wq.transpose(1, 2, 3, 0)        # (cin, dy, dx, cout) fp8
        t = t.reshape(2, 128, 25, 2, 128)   # (kt, p, tau, mt, m)
        g[f"w{i + 1}"] = np.ascontiguousarray(t.reshape(256, 25 * 2 * 128))
    bn = np.zeros((256, 8), np.float32)
    for i in range(4):
        s = np.asarray(inp[f"bn{i}_s"], np.float32) / wscale[i]
        b = np.asarray(inp[f"bn{i}_b"], np.float32)
        if i == 3:
            s = s / 289.0
            b = b / 289.0
        bn[:, 2 * i] = s
        bn[:, 2 * i + 1] = b
    g["bn"] = bn
    return g


_CACHE = {}


def kernel(**inputs):
    _install_ntff_hook()
    from concourse.bass_utils import run_bass_kernel_spmd

    if "nc" not in _CACHE:
        _CACHE["nc"] = build_program(NSAMP)
    nc = _CACHE["nc"]

    shared = prep_weights(inputs)
    xcat = make_xcat(inputs["x"])           # (1024, 17, 256)
    in_maps = []
    for c in range(NCORES):
        xc = xcat[c * NSAMP:(c + 1) * NSAMP]            # (128, 17, 256)
        xcT = np.ascontiguousarray(
            xc.transpose(2, 1, 0).reshape(256, L * NSAMP), dtype=np.float16)
        m = dict(shared)
        m["xcatT"] = xcT
        in_maps.append(m)

    res = run_bass_kernel_spmd(nc, in_maps, core_ids=list(range(NCORES)))
    out = np.zeros((B, DIM, T), np.float32)
    for c in range(NCORES):
        hc = res.results[c]["hout"].T                   # (128, 256)
        out[:, :, c::K] = hc.reshape(B, NW, DIM).transpose(0, 2, 1)
    return out


# revision 23
# speedup vs baseline: 1.0584x; 1.0060x over previous
"""Trainium2 Bass kernel for nn_E2ECompressedGEBDModel (SPoS GEBD model).

Full pipeline per window: 2-layer LSTM (seq len 17) -> per-group cosine
self-similarity (4 groups x 17x17) -> 4x (5x5 conv + BN + ReLU) -> global
mean pool.  Data-parallel over the 1024 independent windows: core c gets
SPoS offset c (128 windows).  Window extraction / scatter-back are pure
index gathers done host-side; all FLOPs run on-device.

conv1-3 run as fp8(e4m3) DoubleRow matmuls (K=256 per instruction, 2x
tensor throughput) with error-feedback (snake-order) weight rounding to
keep quantization error ~0.8%.  Boundary taps only compute their valid
output sub-rectangle (padding-skip).  conv0 uses a 100-row fp16 im2col
built by DMA.

Self-contained: hardcodes all shapes; does not read ./reference.py etc.
"""

import math
import sys
import types

import numpy as np
import ml_dtypes

K = 8
DIM = 256
GROUP = 4
B, T = 4, 256
NW = T // K              # 32 windows per (batch, offset)
L = 2 * K + 1            # 17 sequence length
NCORES = 8
NSAMP = B * NW           # 128 windows per core
H = DIM
GD = DIM // GROUP        # 64

E4 = ml_dtypes.float8_e4m3


def _install_ntff_hook():
    """The agent image's antenv lacks axon_hooks; synthesize it so
    run_bass_kernel_spmd(trace=True) can capture NTFF profiles."""
    if "antenv.axon_hooks" in sys.modules:
        return
    import antenv

    hooks_mod = types.ModuleType("antenv.axon_hooks")
    _hook = [None]
    hooks_mod.set_axon_ntff_profile_hook = lambda h: _hook.__setitem__(0, h)
    hooks_mod.get_axon_ntff_profile_hook = lambda: _hook[0]
    sys.modules["antenv.axon_hooks"] = hooks_mod
    antenv.axon_hooks = hooks_mod
    try:
        from trn_agent_boot.trn_boot import _ntff_profile_via_ctypes

        hooks_mod.set_axon_ntff_profile_hook(
            _ntff_profile_via_ctypes("/opt/axon/libaxon_pjrt.so")
        )
    except Exception:
        pass


# conv tap -> valid output sub-rectangle (r0,r1,c0,c1) and source origin
# (sr,sc) in the unpadded 17x17 input.  tap (dy,dx) contributes to output
# (r,c) from input (r+dy-2, c+dx-2).
def _tap_geom(tau):
    dy, dx = divmod(tau, 5)
    r0, r1 = max(0, 2 - dy), 17 + min(0, 2 - dy)
    c0, c1 = max(0, 2 - dx), 17 + min(0, 2 - dx)
    return r0, r1, c0, c1, max(dy - 2, 0), max(dx - 2, 0)


TAU_ORDER = [12] + [t for t in range(25) if t != 12]   # full-window tap first


def build_program(nsamp=NSAMP, stop_after=None):
    """Build + compile the per-core Bass program (SPMD, identical on all
    cores)."""
    import concourse.bass as bass
    import concourse.mybir as mybir
    import concourse.tile as tile
    from concourse import bacc
    from concourse.masks import make_identity

    dt = mybir.dt
    f32, f16, f8 = dt.float32, dt.float16, dt.float8e4
    AF = mybir.ActivationFunctionType
    PM = mybir.MatmulPerfMode
    NS = nsamp
    ts = bass.ts

    nc = bacc.Bacc("TRN2", target_bir_lowering=False, debug=False,
                   num_devices=NCORES)

    # ---- DRAM I/O --------------------------------------------------------
    d_xcatT = nc.dram_tensor("xcatT", [256, L * NS], f8, kind="ExternalInput")
    d_wc = [nc.dram_tensor(f"wc{l}", [512, 1024], f8, kind="ExternalInput")
            for l in range(2)]
    d_bias = [nc.dram_tensor(f"bias{l}", [1, 1024], f16, kind="ExternalInput")
              for l in range(2)]
    d_w0 = nc.dram_tensor("w0", [100, 2 * 128], f16, kind="ExternalInput")
    d_wconv = [nc.dram_tensor(f"w{l}", [256, 25 * 2 * 128], f8,
                              kind="ExternalInput") for l in (1, 2, 3)]
    d_bn = nc.dram_tensor("bn", [256, 8], f32, kind="ExternalInput")
    d_out = nc.dram_tensor("hout", [256, NS], f32, kind="ExternalOutput")

    with tile.TileContext(nc) as tc:
        wpool = tc.alloc_tile_pool(name="weights", bufs=1)
        work = tc.alloc_tile_pool(name="work", bufs=1)

        # ---- persistent SBUF tensors ------------------------------------
        # [p, layer, io(in/rec), kt-pair, gates] fp8 for DoubleRow
        sb_wc = wpool.tile([128, 2, 2, 2, 1024], f8, tag="wc")
        for l in range(2):
            nc.sync.dma_start(
                out=sb_wc[:, l],
                in_=d_wc[l].ap().rearrange("(a b p) n -> p a b n", a=2, b=2))
        sb_bias = wpool.tile([33, 1024], f16, tag="bias")
        for l in range(2):
            nc.sync.dma_start(out=sb_bias[32 * l:32 * l + 1, :],
                              in_=d_bias[l].ap())
        sb_w0 = wpool.tile([128, 2, 128], f16, tag="w0")
        nc.sync.dma_start(
            out=sb_w0[0:100],
            in_=d_w0.ap().rearrange("p (mt m) -> p mt m", mt=2))
        sb_bn = wpool.tile([128, 2, 8], f32, tag="bn")
        nc.sync.dma_start(out=sb_bn,
                          in_=d_bn.ap().rearrange("(a p) n -> p a n", p=128))
        sb_ones = wpool.tile([33, 128], f16, tag="ones")
        nc.vector.memset(sb_ones[0:1], 1.0)
        nc.vector.memset(sb_ones[32:33], 1.0)
        sb_ident = wpool.tile([128, 128], f32, tag="ident")
        make_identity(nc, sb_ident)

        # LSTM state
        sb_h = work.tile([128, 2, 256], f32, tag="h")        # [s, layer, hid]
        sb_c = work.tile([128, 2, 256], f32, tag="c")
        nc.vector.memset(sb_h, 0.0)
        nc.vector.memset(sb_c, 0.0)
        h0T = work.tile([128, 2, L, NS], f8, tag="h0T")      # [hid_p, kt, t, s]
        hhT = work.tile([128, 2, 2, NS], f8, tag="hhT")      # [hid_p, t%2, kt, s]
        hnT = work.tile([128, 2, NS, L], f8, tag="hnT")      # normalized h1^T
        sb_sig = work.tile([128, 2, 768], f32, tag="sig")
        sb_gg = work.tile([128, 2, 256], f32, tag="gg")
        sb_ig = work.tile([128, 2, 256], f32, tag="ig")
        sb_tc = work.tile([128, 2, 256], f32, tag="tc")
        sb_hn = work.tile([128, 256], f32, tag="hn")
        sb_sq = work.tile([128, 256], f32, tag="sq")
        sb_ss = work.tile([128, 4], f32, tag="ss")
        sb_sr = work.tile([128, 4], f32, tag="sr")
        sb_rn = work.tile([128, 4], f32, tag="rn")
        sb_eps = work.tile([128, 1], f32, tag="eps")
        nc.vector.memset(sb_eps, 1e-8)

        # conv stage persistent buffers (sim front-end runs 4 samples ahead,
        # conv0 2 samples ahead of conv1-3 -> 4-slot buffers)
        zp = work.tile([128, 2, 2, 128], f8, tag="zp")          # [hid, slot, kt, col]
        nc.vector.memset(zp, 0.0)
        s1 = [work.tile([128, 17], f16, tag=f"s1_{i}", name=f"s1_{i}")
              for i in range(4)]
        sim_pad = work.tile([128, 4, 448], f16, tag="sim_pad")  # [g(4), slot, 21*21+7]
        nc.vector.memset(sim_pad, 0.0)
        ic16 = work.tile([128, 4, 368], f16, tag="ic16")        # [(dx,g,dy), slot, 357]
        # unpadded fp8 activations [ch_p, slot, kt, 17*17(+pad)]
        act0 = work.tile([128, 4, 2, 304], f8, tag="act0")
        act = [None,
               work.tile([128, 2, 2, 304], f8, tag="act1", name="act1"),
               work.tile([128, 2, 2, 304], f8, tag="act2", name="act2")]
        scratch = work.tile([128, 2, 289], f32, tag="scratch")
        hout_sb = work.tile([128, 2, NS], f32, tag="hout_sb")
        nc.vector.memset(hout_sb, 0.0)

        # ================= LSTM (2 layers, interleaved) ===================
        xcpool = tc.alloc_tile_pool(name="xcpool", bufs=1)
        sb_xcatT = xcpool.tile([128, 2, L * NS], f8, tag="xcatT")
        nc.sync.dma_start(out=sb_xcatT,
                          in_=d_xcatT.ap().rearrange("(a p) n -> p a n", p=128))
        psz = tc.alloc_tile_pool(name="psz", bufs=2, space="PSUM")
        pst = tc.alloc_tile_pool(name="pst", bufs=4, space="PSUM")

        def lstm_step(layer, t):
            ps = psz.tile([128, 1024], f32, tag="z")
            # bias (k=1 matmul with ones row) — also opens the accum group
            for nh in range(2):
                nc.tensor.matmul(
                    ps[0:NS, ts(nh, 512)],
                    sb_ones[32 * layer:32 * layer + 1, 0:NS],
                    sb_bias[32 * layer:32 * layer + 1, ts(nh, 512)],
                    start=True, stop=False)
            # input contribution (fp8 DoubleRow over the kt pair)
            if layer == 0:
                lhsT = sb_xcatT[:, :, t * NS:(t + 1) * NS]
            else:
                lhsT = h0T[:, :, t, :]
            for nh in range(2):
                nc.tensor.matmul(
                    ps[0:NS, ts(nh, 512)], lhsT,
                    sb_wc[:, layer, 0, :, ts(nh, 512)],
                    start=False, stop=(t == 0 and nh == 1),
                    perf_mode=PM.DoubleRow)
            # recurrent contribution (h_{-1} = 0 -> skip at t=0)
            if t > 0:
                if layer == 0:
                    lhsT = h0T[:, :, t - 1, :]
                else:
                    lhsT = hhT[:, (t - 1) % 2, :, :]
                for nh in range(2):
                    nc.tensor.matmul(
                        ps[0:NS, ts(nh, 512)], lhsT,
                        sb_wc[:, layer, 1, :, ts(nh, 512)],
                        start=False, stop=(nh == 1),
                        perf_mode=PM.DoubleRow)
            # gates: layout [i(0:256) f(256:512) o(512:768) | g(768:1024)]
            sig = sb_sig[0:NS, layer]
            nc.scalar.activation(sig, ps[0:NS, 0:768], AF.Sigmoid)
            nc.scalar.activation(sb_gg[0:NS, layer], ps[0:NS, 768:1024], AF.Tanh)
            c_ = sb_c[0:NS, layer]
            h_ = sb_h[0:NS, layer]
            nc.vector.tensor_mul(sb_ig[0:NS, layer], sig[:, 0:256],
                                 sb_gg[0:NS, layer])
            if t > 0:
                nc.vector.tensor_mul(c_, sig[:, 256:512], c_)
                nc.vector.tensor_add(c_, c_, sb_ig[0:NS, layer])
            else:
                nc.vector.tensor_copy(c_, sb_ig[0:NS, layer])
            nc.scalar.activation(sb_tc[0:NS, layer], c_, AF.Tanh)
            nc.vector.tensor_mul(h_, sig[:, 512:768], sb_tc[0:NS, layer])
            # transpose h_t (raw) for recurrence / next-layer input
            for kt in range(2):
                pt = pst.tile([128, NS], f32, tag="tr")
                nc.tensor.transpose(pt, h_[:, ts(kt, 128)],
                                    sb_ident[0:NS, 0:NS])
                dest = h0T[:, kt, t, :] if layer == 0 else hhT[:, t % 2, kt, :]
                nc.vector.tensor_copy(dest, pt)
            if layer == 1:
                # normalize per similarity group, transpose into hnT
                nc.vector.tensor_mul(sb_sq[0:NS], h_, h_)
                nc.vector.reduce_sum(
                    sb_ss[0:NS],
                    sb_sq[0:NS].rearrange("p (g d) -> p g d", g=4),
                    axis=mybir.AxisListType.X)
                nc.scalar.activation(sb_sr[0:NS], sb_ss[0:NS], AF.Sqrt,
                                     bias=sb_eps[0:NS])
                nc.vector.reciprocal(sb_rn[0:NS], sb_sr[0:NS])
                for g in range(4):
                    nc.vector.tensor_scalar_mul(
                        sb_hn[0:NS, ts(g, GD)], h_[:, ts(g, GD)],
                        sb_rn[0:NS, g:g + 1])
                for kt in range(2):
                    pt = pst.tile([128, NS], f32, tag="tr")
                    nc.tensor.transpose(pt, sb_hn[0:NS, ts(kt, 128)],
                                        sb_ident[0:NS, 0:NS])
                    nc.vector.tensor_copy(hnT[:, kt, :, t], pt)

        # software-pipeline the two layers: L1 runs one step behind L0
        lstm_step(0, 0)
        lstm_step(0, 1)
        for t in range(2, L + 2):
            if t < L:
                lstm_step(0, t)
            lstm_step(1, t - 2)
        pst.release()
        psz.release()
        xcpool.release()

        # ================= similarity + convs, per sample =================
        cwpool = tc.alloc_tile_pool(name="cwpool", bufs=1)
        sb_wconv = cwpool.tile([128, 3, 2, 25, 2, 128], f8, tag="wconv")
        for i in range(3):
            nc.sync.dma_start(
                out=sb_wconv[:, i],
                in_=d_wconv[i].ap().rearrange("(a p) n -> p a n", p=128))
        psim = tc.alloc_tile_pool(name="psim", bufs=1, space="PSUM")
        pcv0 = tc.alloc_tile_pool(name="pcv0", bufs=1, space="PSUM")
        pc_0 = pcv0.tile([128, 2, 512], f32, tag="pc0t")
        pcv = tc.alloc_tile_pool(name="pcv", bufs=1, space="PSUM")
        pc_t = [pcv.tile([128, 2, 512], f32, tag=f"pc{i}", name=f"pc{i}")
                for i in range(2)]

        def sim_stage(s):
            sl = s % 4
            # stationary zp: column block [32g : 32g+17] holds group g's
            # normalized vectors (rows = hidden slice of that group, zeros
            # elsewhere) -> one matmul accumulation group computes all 4
            # group-dot blocks into psum partitions [32g:32g+17].
            for g in range(4):
                kt, ko = g // 2, (g % 2) * GD
                nc.vector.tensor_copy(
                    zp[ko:ko + GD, s % 2, kt, 32 * g:32 * g + 17],
                    hnT[ko:ko + GD, kt, s, :])
            ps = psim.tile([128, 17], f32, tag="psim")
            nc.tensor.matmul(ps, zp[:, s % 2, :, :], hnT[:, :, s, :],
                             start=True, stop=True, perf_mode=PM.DoubleRow)
            nc.vector.tensor_copy(s1[sl], ps)
            # regroup [32g+i, j] -> padded image [g, i(row), j(col)]
            for g in range(4):
                dst = sim_pad[g:g + 1, sl, 0:441].rearrange(
                    "p (r c) -> p r c", c=21)[:, 2:19, 2:19]
                nc.sync.dma_start(out=dst, in_=s1[sl][32 * g:32 * g + 17, :])
            # im2col gather: row (dx,g,dy) = 357-run sim_pad[g, dy*21+dx :][:357]
            # (contiguous run keeps each DMA within the 3-dim balancer limit;
            # the matmul below views it as 17 rows of pitch 21).
            sp = sim_pad[0:4, sl, 0:441]
            for dx in range(5):
                src = bass.AP(tensor=sp.tensor, offset=sp.offset + dx,
                              ap=[sp.ap[0], [21, 5], [1, 357]])
                nc.sync.dma_start(out=ic16[dx * 20:(dx + 1) * 20, sl, 0:357],
                                  in_=src)

        def conv0_stage(s):
            sl = s % 4
            icv = ic16[0:100, sl, 0:357].rearrange(
                "p (r c) -> p r c", c=21)[:, :, 0:17]
            for mt in range(2):
                nc.tensor.matmul(pc_0[:, mt, 0:289],
                                 sb_w0[0:100, mt, :],
                                 icv,
                                 start=True, stop=True)
                nc.scalar.activation(
                    act0[:, sl, mt, 0:289], pc_0[:, mt, 0:289],
                    AF.Relu, scale=sb_bn[:, mt, 0:1], bias=sb_bn[:, mt, 1:2])

        def conv_stage(s, lyr):
            """lyr in 1..3: act[lyr-1] -> act[lyr] (or pooled out).
            fp8 DoubleRow matmuls, padding-skip sub-rectangles."""
            pp = s % 2
            pc = pc_t[pp]
            if lyr == 1:
                src = act0[:, s % 4, :, 0:289].rearrange(
                    "p a (r c) -> p a r c", c=17)
            else:
                src = act[lyr - 1][:, pp, :, 0:289].rearrange(
                    "p a (r c) -> p a r c", c=17)
            for mt in range(2):
                out289 = pc[:, mt, 0:289].rearrange("p (r c) -> p r c", c=17)
                for tau in TAU_ORDER:
                    r0, r1, c0, c1, sr, sc = _tap_geom(tau)
                    nc.tensor.matmul(
                        out289[:, r0:r1, c0:c1],
                        sb_wconv[:, lyr - 1, :, tau, mt, :],
                        src[:, :, sr:sr + (r1 - r0), sc:sc + (c1 - c0)],
                        start=(tau == 12), stop=(tau == TAU_ORDER[-1]),
                        perf_mode=PM.DoubleRow)
                if lyr < 3:
                    nc.scalar.activation(
                        act[lyr][:, pp, mt, 0:289], pc[:, mt, 0:289],
                        AF.Relu, scale=sb_bn[:, mt, 2 * lyr:2 * lyr + 1],
                        bias=sb_bn[:, mt, 2 * lyr + 1:2 * lyr + 2])
                else:
                    nc.scalar.activation(
                        scratch[:, mt], pc[:, mt, 0:289], AF.Relu,
                        scale=sb_bn[:, mt, 6:7], bias=sb_bn[:, mt, 7:8],
                        accum_out=hout_sb[:, mt, s:s + 1])

        # pipelined emission: sims run 4 samples ahead, conv0 2 ahead of the
        # conv1-3 chain; the second conv0 of each window is emitted mid-
        # iteration so the shared pc_0 tile's WAR distance stays long.
        if stop_after != "lstm":
            last = {"sim": 0, "conv0": 0, "conv1": 1, "conv2": 2}.get(
                stop_after, 3)

            def do(stage, s):
                if s < nsamp:
                    stage(s)

            for s in range(4):
                do(sim_stage, s)
            if stop_after != "sim":
                for s in range(2):
                    do(conv0_stage, s)
                for p in range(0, nsamp, 2):
                    pair = [s for s in (p, p + 1) if s < nsamp]
                    do(sim_stage, p + 4)
                    do(sim_stage, p + 5)
                    do(conv0_stage, p + 2)
                    if last >= 1:
                        for s in pair:
                            conv_stage(s, 1)
                    do(conv0_stage, p + 3)
                    for lyr in (2, 3):
                        if lyr > last:
                            break
                        for s in pair:
                            conv_stage(s, lyr)

        nc.sync.dma_start(
            out=d_out.ap().rearrange("(a p) n -> p a n", p=128),
            in_=hout_sb[:, :, 0:NS])

        pcv.release()
        pcv0.release()
        psim.release()
        cwpool.release()
        work.release()
        wpool.release()

    nc.compile()
    return nc


# ======================= host-side preparation ===========================

def make_xcat(x):
    """Window extraction, identical to the reference (pL == T case)."""
    x = np.asarray(x, np.float32)
    lefts, rights, mids = [], [], []
    for offset in range(K):
        s = K - offset
        left = np.concatenate(
            [np.repeat(x[:, :, :1], s, axis=2), x[:, :, :-s]], axis=2)
        r = offset + 1
        right = np.concatenate(
            [x[:, :, r:], np.repeat(x[:, :, -1:], r, axis=2)], axis=2)
        lefts.append(left.reshape(B, DIM, NW, K).transpose(0, 2, 3, 1)
                     .reshape(B * NW, K, DIM))
        rights.append(right.reshape(B, DIM, NW, K).transpose(0, 2, 3, 1)
                      .reshape(B * NW, K, DIM))
        mids.append(x[:, :, offset::K].transpose(0, 2, 1)
                    .reshape(B * NW, 1, DIM))
    left_seq = np.concatenate(lefts, axis=0)
    right_seq = np.concatenate(rights, axis=0)
    mid_seq = np.concatenate(mids, axis=0)
    return np.concatenate([left_seq, mid_seq, right_seq], axis=1)  # (1024,17,256)


def _quant_ef_snake(w, target=224.0):
    """e4m3 quantization with snake-order (dy,dx) error feedback per (o,i).
    Returns (fp8 array same shape, scale s applied: wq ~ w*s)."""
    w = np.asarray(w, np.float32)
    mx = max(np.abs(w).max(), 1e-20)
    s = float(2.0 ** np.floor(np.log2(target / mx)))
    ws = w * s
    out = np.zeros(w.shape, E4)
    carry = np.zeros(w.shape[:2], np.float32)
    idx = []
    for dy in range(5):
        rng = range(5) if dy % 2 == 0 else range(4, -1, -1)
        idx += [(dy, dx) for dx in rng]
    for dy, dx in idx:
        v = ws[:, :, dy, dx] + carry
        qv = v.astype(E4)
        carry = v - qv.astype(np.float32)
        out[:, :, dy, dx] = qv
    return out, s


def prep_weights(inp):
    """Host-side reorder of parameters into the device layouts."""
    g = {}
    perm = np.concatenate([np.arange(0, 256), np.arange(256, 512),
                           np.arange(768, 1024), np.arange(512, 768)])
    for l in range(2):
        wih = np.asarray(inp[f"w_ih{l}"], np.float32)[perm]
        whh = np.asarray(inp[f"w_hh{l}"], np.float32)[perm]
        g[f"wc{l}"] = np.ascontiguousarray(
            np.vstack([wih.T, whh.T])).astype(E4)               # (512,1024)
        g[f"bias{l}"] = np.ascontiguousarray(
            (np.asarray(inp[f"b_ih{l}"], np.float32)
             + np.asarray(inp[f"b_hh{l}"], np.float32))[perm][None, :],
            dtype=np.float16)
    # conv0 layout: rows (dx,g,dy)=100, cols (mt, m)
    w0 = np.asarray(inp["conv0_w"], np.float32)                 # (256,4,5,5)
    t = w0.transpose(3, 1, 2, 0).reshape(100, 256)              # (dx,g,dy),(cout)
    g["w0"] = np.ascontiguousarray(t, dtype=np.float16)
    wscale = np.zeros(4, np.float32)
    wscale[0] = 1.0
    for i, name in enumerate(("conv1_w", "conv2_w", "conv3_w")):
        w = np.asarray(inp[name], np.float32)                   # (256,256,5,5)
        wq, s = _quant_ef_snake(w)
        wscale[i + 1] = s
        t = wq.transpose(1, 2, 3, 0)        # (cin, dy, dx, cout) fp8
        t = t.reshape(2, 128, 25, 2, 128)   # (kt, p, tau, mt, m)
        g[f"w{i + 1}"] = np.ascontiguousarray(t.reshape(256, 25 * 2 * 128))
    bn = np.zeros((256, 8), np.float32)
    for i in range(4):
        s = np.asarray(inp[f"bn{i}_s"], np.float32) / wscale[i]
        b = np.asarray(inp[f"bn{i}_b"], np.float32)
        if i == 3:
            s = s / 289.0
            b = b / 289.0
        bn[:, 2 * i] = s
        bn[:, 2 * i + 1] = b
    g["bn"] = bn
    return g


_CACHE = {}


def kernel(**inputs):
    _install_ntff_hook()
    from concourse.bass_utils import run_bass_kernel_spmd

    if "nc" not in _CACHE:
        _CACHE["nc"] = build_program(NSAMP)
    nc = _CACHE["nc"]

    shared = prep_weights(inputs)
    xcat = make_xcat(inputs["x"])           # (1024, 17, 256)
    in_maps = []
    for c in range(NCORES):
        xc = xcat[c * NSAMP:(c + 1) * NSAMP]            # (128, 17, 256)
        xcT = np.ascontiguousarray(
            xc.transpose(2, 1, 0).reshape(256, L * NSAMP)).astype(E4)
        m = dict(shared)
        m["xcatT"] = xcT
        in_maps.append(m)

    res = run_bass_kernel_spmd(nc, in_maps, core_ids=list(range(NCORES)))
    out = np.zeros((B, DIM, T), np.float32)
    for c in range(NCORES):
        hc = res.results[c]["hout"].T                   # (128, 256)
        out[:, :, c::K] = hc.reshape(B, NW, DIM).transpose(0, 2, 1)
    return out


# revision 25
# speedup vs baseline: 1.0773x; 1.0179x over previous
"""Trainium2 Bass kernel for nn_E2ECompressedGEBDModel (SPoS GEBD model).

Full pipeline per window: 2-layer LSTM (seq len 17) -> per-group cosine
self-similarity (4 groups x 17x17) -> 4x (5x5 conv + BN + ReLU) -> global
mean pool.  Data-parallel over the 1024 independent windows: core c gets
SPoS offset c (128 windows).  Window extraction / scatter-back are pure
index gathers done host-side; all FLOPs run on-device.

conv1-3 run as fp8(e4m3) DoubleRow matmuls (K=256 per instruction, 2x
tensor throughput) with error-feedback (snake-order) weight rounding to
keep quantization error ~0.8%.  Boundary taps only compute their valid
output sub-rectangle (padding-skip).  conv0 uses a 100-row fp16 im2col
built by DMA.

Self-contained: hardcodes all shapes; does not read ./reference.py etc.
"""

import math
import sys
import types

import numpy as np
import ml_dtypes

K = 8
DIM = 256
GROUP = 4
B, T = 4, 256
NW = T // K              # 32 windows per (batch, offset)
L = 2 * K + 1            # 17 sequence length
NCORES = 8
NSAMP = B * NW           # 128 windows per core
H = DIM
GD = DIM // GROUP        # 64

E4 = ml_dtypes.float8_e4m3


def _install_ntff_hook():
    """The agent image's antenv lacks axon_hooks; synthesize it so
    run_bass_kernel_spmd(trace=True) can capture NTFF profiles."""
    if "antenv.axon_hooks" in sys.modules:
        return
    import antenv

    hooks_mod = types.ModuleType("antenv.axon_hooks")
    _hook = [None]
    hooks_mod.set_axon_ntff_profile_hook = lambda h: _hook.__setitem__(0, h)
    hooks_mod.get_axon_ntff_profile_hook = lambda: _hook[0]
    sys.modules["antenv.axon_hooks"] = hooks_mod
    antenv.axon_hooks = hooks_mod
    try:
        from trn_agent_boot.trn_boot import _ntff_profile_via_ctypes

        hooks_mod.set_axon_ntff_profile_hook(
            _ntff_profile_via_ctypes("/opt/axon/libaxon_pjrt.so")
        )
    except Exception:
        pass


# conv tap -> valid output sub-rectangle (r0,r1,c0,c1) and source origin
# (sr,sc) in the unpadded 17x17 input.  tap (dy,dx) contributes to output
# (r,c) from input (r+dy-2, c+dx-2).
def _tap_geom(tau):
    dy, dx = divmod(tau, 5)
    r0, r1 = max(0, 2 - dy), 17 + min(0, 2 - dy)
    c0, c1 = max(0, 2 - dx), 17 + min(0, 2 - dx)
    return r0, r1, c0, c1, max(dy - 2, 0), max(dx - 2, 0)


TAU_ORDER = [12] + [t for t in range(25) if t != 12]   # full-window tap first


def build_program(nsamp=NSAMP, stop_after=None):
    """Build + compile the per-core Bass program (SPMD, identical on all
    cores)."""
    import concourse.bass as bass
    import concourse.mybir as mybir
    import concourse.tile as tile
    from concourse import bacc
    from concourse.masks import make_identity

    dt = mybir.dt
    f32, f16, f8 = dt.float32, dt.float16, dt.float8e4
    AF = mybir.ActivationFunctionType
    PM = mybir.MatmulPerfMode
    NS = nsamp
    ts = bass.ts

    nc = bacc.Bacc("TRN2", target_bir_lowering=False, debug=False,
                   num_devices=NCORES)

    # ---- DRAM I/O --------------------------------------------------------
    d_xcatT = nc.dram_tensor("xcatT", [256, L * NS], f8, kind="ExternalInput")
    d_wc = [nc.dram_tensor(f"wc{l}", [512, 1024], f8, kind="ExternalInput")
            for l in range(2)]
    d_bias = [nc.dram_tensor(f"bias{l}", [1, 1024], f16, kind="ExternalInput")
              for l in range(2)]
    d_w0 = nc.dram_tensor("w0", [100, 2 * 128], f16, kind="ExternalInput")
    d_wconv = [nc.dram_tensor(f"w{l}", [256, 25 * 2 * 128], f8,
                              kind="ExternalInput") for l in (1, 2, 3)]
    d_bn = nc.dram_tensor("bn", [256, 8], f32, kind="ExternalInput")
    d_out = nc.dram_tensor("hout", [256, NS], f32, kind="ExternalOutput")

    with tile.TileContext(nc) as tc:
        wpool = tc.alloc_tile_pool(name="weights", bufs=1)
        work = tc.alloc_tile_pool(name="work", bufs=1)

        # ---- persistent SBUF tensors ------------------------------------
        # [p, layer, io(in/rec), kt-pair, gates] fp8 for DoubleRow
        sb_wc = wpool.tile([128, 2, 2, 2, 1024], f8, tag="wc")
        for l in range(2):
            nc.sync.dma_start(
                out=sb_wc[:, l],
                in_=d_wc[l].ap().rearrange("(a b p) n -> p a b n", a=2, b=2))
        sb_bias = wpool.tile([33, 1024], f16, tag="bias")
        for l in range(2):
            nc.sync.dma_start(out=sb_bias[32 * l:32 * l + 1, :],
                              in_=d_bias[l].ap())
        sb_w0 = wpool.tile([128, 2, 128], f16, tag="w0")
        nc.sync.dma_start(
            out=sb_w0[0:100],
            in_=d_w0.ap().rearrange("p (mt m) -> p mt m", mt=2))
        sb_bn = wpool.tile([128, 2, 8], f32, tag="bn")
        nc.sync.dma_start(out=sb_bn,
                          in_=d_bn.ap().rearrange("(a p) n -> p a n", p=128))
        sb_ones = wpool.tile([33, 128], f16, tag="ones")
        nc.vector.memset(sb_ones[0:1], 1.0)
        nc.vector.memset(sb_ones[32:33], 1.0)
        sb_ident = wpool.tile([128, 128], f32, tag="ident")
        make_identity(nc, sb_ident)

        # LSTM state
        sb_h = work.tile([128, 2, 256], f32, tag="h")        # [s, layer, hid]
        sb_c = work.tile([128, 2, 256], f32, tag="c")
        nc.vector.memset(sb_h, 0.0)
        nc.vector.memset(sb_c, 0.0)
        h0T = work.tile([128, 2, L, NS], f8, tag="h0T")      # [hid_p, kt, t, s]
        hhT = work.tile([128, 2, 2, NS], f8, tag="hhT")      # [hid_p, t%2, kt, s]
        hnT = work.tile([128, 2, NS, L], f8, tag="hnT")      # normalized h1^T
        sb_sig = work.tile([128, 2, 768], f32, tag="sig")
        sb_gg = work.tile([128, 2, 256], f32, tag="gg")
        sb_ig = work.tile([128, 2, 256], f32, tag="ig")
        sb_tc = work.tile([128, 2, 256], f32, tag="tc")
        sb_hn = work.tile([128, 256], f32, tag="hn")
        sb_sq = work.tile([128, 256], f32, tag="sq")
        sb_ss = work.tile([128, 4], f32, tag="ss")
        sb_sr = work.tile([128, 4], f32, tag="sr")
        sb_rn = work.tile([128, 4], f32, tag="rn")
        sb_eps = work.tile([128, 1], f32, tag="eps")
        nc.vector.memset(sb_eps, 1e-8)

        # conv stage persistent buffers (sim front-end runs 4 samples ahead,
        # conv0 2 samples ahead of conv1-3 -> 4-slot buffers)
        zp = work.tile([128, 2, 2, 128], f8, tag="zp")          # [hid, slot, kt, col]
        nc.vector.memset(zp, 0.0)
        s1 = [work.tile([128, 17], f16, tag=f"s1_{i}", name=f"s1_{i}")
              for i in range(4)]
        sim_pad = work.tile([128, 4, 448], f16, tag="sim_pad")  # [g(4), slot, 21*21+7]
        nc.vector.memset(sim_pad, 0.0)
        ic16 = work.tile([128, 4, 368], f16, tag="ic16")        # [(dx,g,dy), slot, 357]
        # unpadded fp8 activations [ch_p, slot, kt, 17*17(+pad)]
        act0 = work.tile([128, 4, 2, 304], f8, tag="act0")
        act = [None,
               work.tile([128, 2, 2, 304], f8, tag="act1", name="act1"),
               work.tile([128, 2, 2, 304], f8, tag="act2", name="act2")]
        scratch = work.tile([128, 2, 289], f32, tag="scratch")
        hout_sb = work.tile([128, 2, NS], f32, tag="hout_sb")
        nc.vector.memset(hout_sb, 0.0)

        # ================= LSTM (2 layers, interleaved) ===================
        xcpool = tc.alloc_tile_pool(name="xcpool", bufs=1)
        sb_xcatT = xcpool.tile([128, 2, L * NS], f8, tag="xcatT")
        nc.sync.dma_start(out=sb_xcatT,
                          in_=d_xcatT.ap().rearrange("(a p) n -> p a n", p=128))
        psz = tc.alloc_tile_pool(name="psz", bufs=3, space="PSUM")
        pst = tc.alloc_tile_pool(name="pst", bufs=2, space="PSUM")
        zt = {}

        def lstm_A(layer, t):
            """bias + input matmuls: open the z accumulation group."""
            ps = psz.tile([128, 1024], f32, tag="z")
            zt[(layer, t)] = ps
            for nh in range(2):
                nc.tensor.matmul(
                    ps[0:NS, ts(nh, 512)],
                    sb_ones[32 * layer:32 * layer + 1, 0:NS],
                    sb_bias[32 * layer:32 * layer + 1, ts(nh, 512)],
                    start=True, stop=False)
            if layer == 0:
                lhsT = sb_xcatT[:, :, t * NS:(t + 1) * NS]
            else:
                lhsT = h0T[:, :, t, :]
            for nh in range(2):
                nc.tensor.matmul(
                    ps[0:NS, ts(nh, 512)], lhsT,
                    sb_wc[:, layer, 0, :, ts(nh, 512)],
                    start=False, stop=(t == 0 and nh == 1),
                    perf_mode=PM.DoubleRow)

        def lstm_T(layer, t):
            """transpose h_t for recurrence / next-layer input."""
            h_ = sb_h[0:NS, layer]
            for kt in range(2):
                pt = pst.tile([128, NS], f32, tag="tr")
                nc.tensor.transpose(pt, h_[:, ts(kt, 128)],
                                    sb_ident[0:NS, 0:NS])
                dest = h0T[:, kt, t, :] if layer == 0 else hhT[:, t % 2, kt, :]
                nc.vector.tensor_copy(dest, pt)

        def lstm_R(layer, t):
            """recurrent matmuls: close the z group (h_{-1}=0 -> skip t=0)."""
            ps = zt[(layer, t)]
            if layer == 0:
                lhsT = h0T[:, :, t - 1, :]
            else:
                lhsT = hhT[:, (t - 1) % 2, :, :]
            for nh in range(2):
                nc.tensor.matmul(
                    ps[0:NS, ts(nh, 512)], lhsT,
                    sb_wc[:, layer, 1, :, ts(nh, 512)],
                    start=False, stop=(nh == 1),
                    perf_mode=PM.DoubleRow)

        def lstm_G(layer, t):
            """gates: layout [i(0:256) f(256:512) o(512:768) | g(768:1024)]"""
            ps = zt.pop((layer, t))
            sig = sb_sig[0:NS, layer]
            nc.scalar.activation(sig, ps[0:NS, 0:768], AF.Sigmoid)
            nc.scalar.activation(sb_gg[0:NS, layer], ps[0:NS, 768:1024], AF.Tanh)
            c_ = sb_c[0:NS, layer]
            h_ = sb_h[0:NS, layer]
            nc.vector.tensor_mul(sb_ig[0:NS, layer], sig[:, 0:256],
                                 sb_gg[0:NS, layer])
            if t > 0:
                nc.vector.tensor_mul(c_, sig[:, 256:512], c_)
                nc.vector.tensor_add(c_, c_, sb_ig[0:NS, layer])
            else:
                nc.vector.tensor_copy(c_, sb_ig[0:NS, layer])
            nc.scalar.activation(sb_tc[0:NS, layer], c_, AF.Tanh)
            nc.vector.tensor_mul(h_, sig[:, 512:768], sb_tc[0:NS, layer])
            if layer == 1:
                # normalize per similarity group (hnT transposes emitted later)
                nc.vector.tensor_mul(sb_sq[0:NS], h_, h_)
                nc.vector.reduce_sum(
                    sb_ss[0:NS],
                    sb_sq[0:NS].rearrange("p (g d) -> p g d", g=4),
                    axis=mybir.AxisListType.X)
                nc.scalar.activation(sb_sr[0:NS], sb_ss[0:NS], AF.Sqrt,
                                     bias=sb_eps[0:NS])
                nc.vector.reciprocal(sb_rn[0:NS], sb_sr[0:NS])
                for g in range(4):
                    nc.vector.tensor_scalar_mul(
                        sb_hn[0:NS, ts(g, GD)], h_[:, ts(g, GD)],
                        sb_rn[0:NS, g:g + 1])

        def lstm_N(t):
            """transpose normalized h1 into hnT (off the critical path)."""
            for kt in range(2):
                pt = pst.tile([128, NS], f32, tag="tr")
                nc.tensor.transpose(pt, sb_hn[0:NS, ts(kt, 128)],
                                    sb_ident[0:NS, 0:NS])
                nc.vector.tensor_copy(hnT[:, kt, :, t], pt)

        # Two-layer software pipeline, emitted so that independent matmuls
        # (A of both layers) precede the dependency-blocked transposes and
        # recurrent matmuls in the tensor queue.
        def valid0(t):
            return 0 <= t < L

        for t in range(0, L + 4):
            if valid0(t):
                lstm_A(0, t)
            if valid0(t - 2):
                lstm_A(1, t - 2)
            if valid0(t - 1):
                lstm_T(0, t - 1)          # h0(t-1) -> feeds R0(t), A1(t-1)
            if valid0(t - 3):
                lstm_T(1, t - 3)
                lstm_N(t - 3)
            if t > 0 and valid0(t):
                lstm_R(0, t)
            if t - 2 > 0 and valid0(t - 2):
                lstm_R(1, t - 2)
            if valid0(t):
                lstm_G(0, t)
            if valid0(t - 2):
                lstm_G(1, t - 2)
        pst.release()
        psz.release()
        xcpool.release()

        # ================= similarity + convs, per sample =================
        cwpool = tc.alloc_tile_pool(name="cwpool", bufs=1)
        sb_wconv = cwpool.tile([128, 3, 2, 25, 2, 128], f8, tag="wconv")
        for i in range(3):
            nc.sync.dma_start(
                out=sb_wconv[:, i],
                in_=d_wconv[i].ap().rearrange("(a p) n -> p a n", p=128))
        psim = tc.alloc_tile_pool(name="psim", bufs=1, space="PSUM")
        pcv0 = tc.alloc_tile_pool(name="pcv0", bufs=1, space="PSUM")
        pc_0 = pcv0.tile([128, 2, 512], f32, tag="pc0t")
        pcv = tc.alloc_tile_pool(name="pcv", bufs=1, space="PSUM")
        pc_t = [pcv.tile([128, 2, 512], f32, tag=f"pc{i}", name=f"pc{i}")
                for i in range(2)]

        def sim_stage(s):
            sl = s % 4
            # stationary zp: column block [32g : 32g+17] holds group g's
            # normalized vectors (rows = hidden slice of that group, zeros
            # elsewhere) -> one matmul accumulation group computes all 4
            # group-dot blocks into psum partitions [32g:32g+17].
            for g in range(4):
                kt, ko = g // 2, (g % 2) * GD
                nc.vector.tensor_copy(
                    zp[ko:ko + GD, s % 2, kt, 32 * g:32 * g + 17],
                    hnT[ko:ko + GD, kt, s, :])
            ps = psim.tile([128, 17], f32, tag="psim")
            nc.tensor.matmul(ps, zp[:, s % 2, :, :], hnT[:, :, s, :],
                             start=True, stop=True, perf_mode=PM.DoubleRow)
            nc.vector.tensor_copy(s1[sl], ps)
            # regroup [32g+i, j] -> padded image [g, i(row), j(col)]
            for g in range(4):
                dst = sim_pad[g:g + 1, sl, 0:441].rearrange(
                    "p (r c) -> p r c", c=21)[:, 2:19, 2:19]
                nc.sync.dma_start(out=dst, in_=s1[sl][32 * g:32 * g + 17, :])
            # im2col gather: row (dx,g,dy) = 357-run sim_pad[g, dy*21+dx :][:357]
            # (contiguous run keeps each DMA within the 3-dim balancer limit;
            # the matmul below views it as 17 rows of pitch 21).
            sp = sim_pad[0:4, sl, 0:441]
            for dx in range(5):
                src = bass.AP(tensor=sp.tensor, offset=sp.offset + dx,
                              ap=[sp.ap[0], [21, 5], [1, 357]])
                nc.sync.dma_start(out=ic16[dx * 20:(dx + 1) * 20, sl, 0:357],
                                  in_=src)

        def conv0_stage(s):
            sl = s % 4
            icv = ic16[0:100, sl, 0:357].rearrange(
                "p (r c) -> p r c", c=21)[:, :, 0:17]
            for mt in range(2):
                nc.tensor.matmul(pc_0[:, mt, 0:289],
                                 sb_w0[0:100, mt, :],
                                 icv,
                                 start=True, stop=True)
                nc.scalar.activation(
                    act0[:, sl, mt, 0:289], pc_0[:, mt, 0:289],
                    AF.Relu, scale=sb_bn[:, mt, 0:1], bias=sb_bn[:, mt, 1:2])

        def conv_stage(s, lyr):
            """lyr in 1..3: act[lyr-1] -> act[lyr] (or pooled out).
            fp8 DoubleRow matmuls, padding-skip sub-rectangles."""
            pp = s % 2
            pc = pc_t[pp]
            if lyr == 1:
                src = act0[:, s % 4, :, 0:289].rearrange(
                    "p a (r c) -> p a r c", c=17)
            else:
                src = act[lyr - 1][:, pp, :, 0:289].rearrange(
                    "p a (r c) -> p a r c", c=17)
            for mt in range(2):
                out289 = pc[:, mt, 0:289].rearrange("p (r c) -> p r c", c=17)
                for tau in TAU_ORDER:
                    r0, r1, c0, c1, sr, sc = _tap_geom(tau)
                    nc.tensor.matmul(
                        out289[:, r0:r1, c0:c1],
                        sb_wconv[:, lyr - 1, :, tau, mt, :],
                        src[:, :, sr:sr + (r1 - r0), sc:sc + (c1 - c0)],
                        start=(tau == 12), stop=(tau == TAU_ORDER[-1]),
                        perf_mode=PM.DoubleRow)
                if lyr < 3:
                    nc.scalar.activation(
                        act[lyr][:, pp, mt, 0:289], pc[:, mt, 0:289],
                        AF.Relu, scale=sb_bn[:, mt, 2 * lyr:2 * lyr + 1],
                        bias=sb_bn[:, mt, 2 * lyr + 1:2 * lyr + 2])
                else:
                    nc.scalar.activation(
                        scratch[:, mt], pc[:, mt, 0:289], AF.Relu,
                        scale=sb_bn[:, mt, 6:7], bias=sb_bn[:, mt, 7:8],
                        accum_out=hout_sb[:, mt, s:s + 1])

        # pipelined emission: sims run 4 samples ahead, conv0 2 ahead of the
        # conv1-3 chain; the second conv0 of each window is emitted mid-
        # iteration so the shared pc_0 tile's WAR distance stays long.
        if stop_after != "lstm":
            last = {"sim": 0, "conv0": 0, "conv1": 1, "conv2": 2}.get(
                stop_after, 3)

            def do(stage, s):
                if s < nsamp:
                    stage(s)

            for s in range(4):
                do(sim_stage, s)
            if stop_after != "sim":
                for s in range(2):
                    do(conv0_stage, s)
                for p in range(0, nsamp, 2):
                    pair = [s for s in (p, p + 1) if s < nsamp]
                    do(sim_stage, p + 4)
                    do(sim_stage, p + 5)
                    do(conv0_stage, p + 2)
                    if last >= 1:
                        for s in pair:
                            conv_stage(s, 1)
                    do(conv0_stage, p + 3)
                    for lyr in (2, 3):
                        if lyr > last:
                            break
                        for s in pair:
                            conv_stage(s, lyr)

        nc.sync.dma_start(
            out=d_out.ap().rearrange("(a p) n -> p a n", p=128),
            in_=hout_sb[:, :, 0:NS])

        pcv.release()
        pcv0.release()
        psim.release()
        cwpool.release()
        work.release()
        wpool.release()

    nc.compile()
    return nc


# ======================= host-side preparation ===========================

def make_xcat(x):
    """Window extraction, identical to the reference (pL == T case)."""
    x = np.asarray(x, np.float32)
    lefts, rights, mids = [], [], []
    for offset in range(K):
        s = K - offset
        left = np.concatenate(
            [np.repeat(x[:, :, :1], s, axis=2), x[:, :, :-s]], axis=2)
        r = offset + 1
        right = np.concatenate(
            [x[:, :, r:], np.repeat(x[:, :, -1:], r, axis=2)], axis=2)
        lefts.append(left.reshape(B, DIM, NW, K).transpose(0, 2, 3, 1)
                     .reshape(B * NW, K, DIM))
        rights.append(right.reshape(B, DIM, NW, K).transpose(0, 2, 3, 1)
                      .reshape(B * NW, K, DIM))
        mids.append(x[:, :, offset::K].transpose(0, 2, 1)
                    .reshape(B * NW, 1, DIM))
    left_seq = np.concatenate(lefts, axis=0)
    right_seq = np.concatenate(rights, axis=0)
    mid_seq = np.concatenate(mids, axis=0)
    return np.concatenate([left_seq, mid_seq, right_seq], axis=1)  # (1024,17,256)


def _quant_ef_snake(w, target=224.0):
    """e4m3 quantization with snake-order (dy,dx) error feedback per (o,i).
    Returns (fp8 array same shape, scale s applied: wq ~ w*s)."""
    w = np.asarray(w, np.float32)
    mx = max(np.abs(w).max(), 1e-20)
    s = float(2.0 ** np.floor(np.log2(target / mx)))
    ws = w * s
    out = np.zeros(w.shape, E4)
    carry = np.zeros(w.shape[:2], np.float32)
    idx = []
    for dy in range(5):
        rng = range(5) if dy % 2 == 0 else range(4, -1, -1)
        idx += [(dy, dx) for dx in rng]
    for dy, dx in idx:
        v = ws[:, :, dy, dx] + carry
        qv = v.astype(E4)
        carry = v - qv.astype(np.float32)
        out[:, :, dy, dx] = qv
    return out, s


def prep_weights(inp):
    """Host-side reorder of parameters into the device layouts."""
    g = {}
    perm = np.concatenate([np.arange(0, 256), np.arange(256, 512),
                           np.arange(768, 1024), np.arange(512, 768)])
    for l in range(2):
        wih = np.asarray(inp[f"w_ih{l}"], np.float32)[perm]
        whh = np.asarray(inp[f"w_hh{l}"], np.float32)[perm]
        g[f"wc{l}"] = np.ascontiguousarray(
            np.vstack([wih.T, whh.T])).astype(E4)               # (512,1024)
        g[f"bias{l}"] = np.ascontiguousarray(
            (np.asarray(inp[f"b_ih{l}"], np.float32)
             + np.asarray(inp[f"b_hh{l}"], np.float32))[perm][None, :],
            dtype=np.float16)
    # conv0 layout: rows (dx,g,dy)=100, cols (mt, m)
    w0 = np.asarray(inp["conv0_w"], np.float32)                 # (256,4,5,5)
    t = w0.transpose(3, 1, 2, 0).reshape(100, 256)              # (dx,g,dy),(cout)
    g["w0"] = np.ascontiguousarray(t, dtype=np.float16)
    wscale = np.zeros(4, np.float32)
    wscale[0] = 1.0
    for i, name in enumerate(("conv1_w", "conv2_w", "conv3_w")):
        w = np.asarray(inp[name], np.float32)                   # (256,256,5,5)
        wq, s = _quant_ef_snake(w)
        wscale[i + 1] = s
        t = wq.transpose(1, 2, 3, 0)        # (cin, dy, dx, cout) fp8
        t = t.reshape(2, 128, 25, 2, 128)   # (kt, p, tau, mt, m)
        g[f"w{i + 1}"] = np.ascontiguousarray(t.reshape(256, 25 * 2 * 128))
    bn = np.zeros((256, 8), np.float32)
    for i in range(4):
        s = np.asarray(inp[f"bn{i}_s"], np.float32) / wscale[i]
        b = np.asarray(inp[f"bn{i}_b"], np.float32)
        if i == 3:
            s = s / 289.0
            b = b / 289.0
        bn[:, 2 * i] = s
        bn[:, 2 * i + 1] = b
    g["bn"] = bn
    return g


_CACHE = {}


def kernel(**inputs):
    _install_ntff_hook()
    from concourse.bass_utils import run_bass_kernel_spmd

    if "nc" not in _CACHE:
        _CACHE["nc"] = build_program(NSAMP)
    nc = _CACHE["nc"]

    shared = prep_weights(inputs)
    xcat = make_xcat(inputs["x"])           # (1024, 17, 256)
    in_maps = []
    for c in range(NCORES):
        xc = xcat[c * NSAMP:(c + 1) * NSAMP]            # (128, 17, 256)
        xcT = np.ascontiguousarray(
            xc.transpose(2, 1, 0).reshape(256, L * NSAMP)).astype(E4)
        m = dict(shared)
        m["xcatT"] = xcT
        in_maps.append(m)

    res = run_bass_kernel_spmd(nc, in_maps, core_ids=list(range(NCORES)))
    out = np.zeros((B, DIM, T), np.float32)
    for c in range(NCORES):
        hc = res.results[c]["hout"].T                   # (128, 256)
        out[:, :, c::K] = hc.reshape(B, NW, DIM).transpose(0, 2, 1)
    return out
